# revision 8
# baseline (speedup 1.0000x reference)
"""CalderaLinear Trainium2 kernel (v2: fp8 DoubleRow main GEMM).

Computes out = x @ dequant(q).T + (x @ dequant(r).T) @ dequant(l).T + bias
with groupwise (group=128) dequantization, distributed over 8 NeuronCores
by sharding tokens (batch*seq) 8 ways and replicating the weights.

Numerics: the output scale is dominated by the low-rank path (|out| up to
~1.4e6 vs main-path contributions ~2e3), so the x@q.T GEMM runs in fp8
(e4m3) with DoubleRow perf mode (2 fp8 weights/PE cell) at ~1.5-2x bf16
throughput while adding negligible absolute error. The low-rank path
(x@r.T and xr@l.T) stays bf16 with fp32 PSUM accumulation.

Host does layout only: dequant-multiply + transpose + fp8/bf16 casts and
token sharding; all 309 GFLOP of matmul work runs on device.

Device per core (1024 tokens):
  phase 1: stream x.T (bf16, 4 quarters), cast to fp8 copy, and compute
           xr.T = (x @ r_deq.T).T via 128 bf16 matmuls.
  phase 2: for each pair of 512-wide output blocks: stream q fp8 block,
           16 DoubleRow matmuls (256k x 128t x 512o) per psum tile pair
           sharing each stationary x tile, + 2 bf16 low-rank matmuls into
           the same psum group; bias added during the PSUM->SBUF copy.
"""

import os
import sys

import numpy as np
import ml_dtypes

for _p in ("/opt/trn_rl_repo",):
    if _p not in sys.path and os.path.isdir(_p):
        sys.path.insert(0, _p)

import concourse.bass as bass
import concourse.mybir as mybir
import concourse.tile as tile
from concourse import bacc
from concourse.bass_utils import run_bass_kernel_spmd

BF16 = mybir.dt.bfloat16
F32 = mybir.dt.float32
FP8 = mybir.dt.float8e4
NP_FP8 = ml_dtypes.float8_e4m3
NP_BF16 = ml_dtypes.bfloat16

P = 128  # partitions / dequant group size
N_CORES = 8

# Full problem shape (hardcoded per contest contract).
B, S, D_IN, D_OUT, RANK = 4, 2048, 4096, 4096, 256
N_TOK = B * S          # 8192
T = N_TOK // N_CORES   # 1024 tokens per core
G = D_IN // P          # 32 k-chunks
GP = G // 2            # 16 k-pair-chunks (DoubleRow)
OBW = 512              # output block width
NOB = D_OUT // OBW     # 8 output blocks
RC = RANK // P         # 2 rank chunks
XQ = 8                 # x streamed in XQ chunks
GQ = G // XQ           # k-chunks per x chunk


def caldera_kernel(tc, out, xT_d, q8_d, rT_d, lT_d, biasr_d):
    """One core. DRAM tensors:
    xT_d    [128, G, T]        bf16  xT[p,g,t] = x[t, g*128+p]
    q8_d    [NOB, 128, GP*2*OBW] fp8 q8[ob,p,gp,i,o] = qdeq[(2gp+i)*128+p, ob*512+o]
    rT_d    [128, G, RANK]     bf16  rT[p,g,r] = rdeq[r, g*128+p]
    lT_d    [128, RC, D_OUT]   bf16  lT[p,c,o] = ldeq[o, c*128+p]
    biasr_d [128, D_OUT]       f32   bias replicated over partitions
    out     [T, D_OUT]         f32
    """
    nc = tc.nc
    DR = mybir.MatmulPerfMode.DoubleRow
    NT = T // P  # 8 token tiles

    with tc.tile_pool(name="const", bufs=1) as constp, \
         tc.tile_pool(name="xq", bufs=2) as xqp, \
         tc.tile_pool(name="qs", bufs=4) as qsp, \
         tc.tile_pool(name="outp", bufs=4) as outp, \
         tc.tile_pool(name="ps", bufs=6, space="PSUM") as psp:

        # ---- resident tensors ----
        x8 = constp.tile([P, G, T], FP8)        # fp8 copy of xT
        rT = constp.tile([P, G, RANK], BF16)
        lT = constp.tile([P, RC, D_OUT], BF16)
        biasr = constp.tile([P, D_OUT], BF16)
        xrT = constp.tile([P, RC, T], BF16)     # xr.T chunks

        # Startup-critical DMAs first (emission order sets fetch priority):
        # first x chunk -> r weights -> first q pair; everything else after.
        xr_ps = [
            psp.tile([P, OBW], F32, tag="ps", name=f"xrps{i}")
            for i in range(4)
        ]
        xtiles = []
        for q4 in range(XQ):
            xq = xqp.tile([P, GQ, T], BF16, tag="xq", name=f"xq{q4}")
            xtiles.append(xq)

        nc.sync.dma_start(out=xtiles[0][:], in_=xT_d[:, 0:GQ, :])
        nc.sync.dma_start(out=rT[:], in_=rT_d[:])
        for q4 in range(1, XQ):
            nc.sync.dma_start(
                out=xtiles[q4][:], in_=xT_d[:, q4 * GQ:(q4 + 1) * GQ, :]
            )

        qtiles = []
        for ob in range(NOB):
            qt = qsp.tile([P, GP, 2, OBW], FP8, tag="q8b", name=f"q8b{ob}")
            qtiles.append(qt)

        def fetch_q(ob):
            nc.sync.dma_start(
                out=qtiles[ob][:].rearrange("p a b c -> p (a b c)"),
                in_=q8_d[ob],
            )

        fetch_q(0)
        nc.sync.dma_start(out=lT[:], in_=lT_d[:])
        fetch_q(1)
        nc.sync.dma_start(out=biasr[:], in_=biasr_d[:])

        # ---- x chunks: cast to fp8, feed xr matmuls ----
        for q4 in range(XQ):
            xq = xtiles[q4]
            # fp8 cast (vector engine), one op per chunk
            nc.vector.tensor_copy(
                out=x8[:, q4 * GQ:(q4 + 1) * GQ, :], in_=xq[:]
            )
            # xr.T accumulation: psum[rb*2+th] += rT-chunk.T @ xq-chunk
            for gl in range(GQ):
                g = q4 * GQ + gl
                for rb in range(RC):
                    for th in range(2):
                        nc.tensor.matmul(
                            xr_ps[rb * 2 + th][:],
                            lhsT=rT[:, g, rb * P:(rb + 1) * P],
                            rhs=xq[:, gl, th * OBW:(th + 1) * OBW],
                            start=(g == 0),
                            stop=(g == G - 1),
                        )
        for ob in range(2, NOB):
            fetch_q(ob)
        for rb in range(RC):
            for th in range(2):
                nc.scalar.copy(
                    xrT[:, rb, th * OBW:(th + 1) * OBW],
                    xr_ps[rb * 2 + th][:],
                )

        # ---- main: fp8 DoubleRow + bf16 low-rank into same psum group ----
        for ob in range(NOB):
            qt = qtiles[ob]
            for t in range(NT):
                ps = psp.tile([P, OBW], F32, tag="ps", name=f"ps{ob}_{t}")
                for gp in range(GP):
                    nc.tensor.matmul(
                        ps[:],
                        lhsT=x8[:, 2 * gp:2 * gp + 2, t * P:(t + 1) * P],
                        rhs=qt[:, gp],
                        start=(gp == 0), stop=False, perf_mode=DR,
                    )
                for rb in range(RC):
                    nc.tensor.matmul(
                        ps[:], lhsT=xrT[:, rb, t * P:(t + 1) * P],
                        rhs=lT[:, rb, ob * OBW:(ob + 1) * OBW],
                        start=False, stop=(rb == RC - 1),
                    )
                ot = outp.tile([P, OBW], F32, tag="ot")
                nc.vector.tensor_tensor(
                    out=ot[:], in0=ps[:],
                    in1=biasr[:, ob * OBW:(ob + 1) * OBW],
                    op=mybir.AluOpType.add,
                )
                nc.sync.dma_start(
                    out=out[t * P:(t + 1) * P, ob * OBW:(ob + 1) * OBW],
                    in_=ot[:],
                )


def build_nc():
    nc = bacc.Bacc("TRN2", target_bir_lowering=False, debug=False)
    xT_d = nc.dram_tensor("xT", [P, G, T], BF16, kind="ExternalInput").ap()
    q8_d = nc.dram_tensor(
        "q8", [NOB, P, GP * 2 * OBW], FP8, kind="ExternalInput"
    ).ap()
    rT_d = nc.dram_tensor("rT", [P, G, RANK], BF16, kind="ExternalInput").ap()
    lT_d = nc.dram_tensor("lT", [P, RC, D_OUT], BF16, kind="ExternalInput").ap()
    biasr_d = nc.dram_tensor(
        "biasr", [P, D_OUT], BF16, kind="ExternalInput"
    ).ap()
    out = nc.dram_tensor("out", [T, D_OUT], F32, kind="ExternalOutput").ap()
    with tile.TileContext(nc) as tc:
        caldera_kernel(tc, out, xT_d, q8_d, rT_d, lT_d, biasr_d)
    nc.compile()
    return nc


def _dequant(vals, scales):
    rows, cols = vals.shape
    g = cols // P
    v = vals.astype(np.float32).reshape(rows, g, P) * scales[:, :, None]
    return v.reshape(rows, cols)


def make_in_maps(x, q_values, q_scales, l_values, l_scales, r_values, r_scales,
                 bias):
    # q: dequant -> [k, o] transpose -> fp8, packed per 512-col block:
    # q8[ob, p, (gp, i, o)] = qdeq[(2gp+i)*128+p, ob*512+o]
    qdeq = _dequant(np.asarray(q_values), np.asarray(q_scales))  # [o, k]
    qT = np.ascontiguousarray(qdeq.T).astype(NP_FP8)             # [k, o]
    # [k, o] -> (gp, i, p, o) -> [p, gp, i, ob, o'] -> [ob, p, gp*i*o']
    q8 = qT.reshape(GP, 2, P, NOB, OBW).transpose(3, 2, 0, 1, 4)
    q8 = np.ascontiguousarray(q8).reshape(NOB, P, GP * 2 * OBW)

    rdeq = _dequant(np.asarray(r_values), np.asarray(r_scales))  # [r, k]
    rT = np.ascontiguousarray(
        rdeq.T.reshape(G, P, RANK).transpose(1, 0, 2)
    ).astype(NP_BF16)                                            # [p, g, r]

    ldeq = _dequant(np.asarray(l_values), np.asarray(l_scales))  # [o, r]
    lT = np.ascontiguousarray(
        ldeq.T.reshape(RC, P, D_OUT).transpose(1, 0, 2)
    ).astype(NP_BF16)                                            # [p, c, o]

    biasr = np.ascontiguousarray(
        np.broadcast_to(
            np.asarray(bias, dtype=np.float32).astype(NP_BF16), (P, D_OUT)
        )
    )

    xf = np.asarray(x, dtype=np.float32).reshape(N_TOK, D_IN)
    in_maps = []
    for i in range(N_CORES):
        xs = xf[i * T:(i + 1) * T]                               # [t, k]
        xT = np.ascontiguousarray(
            xs.T.reshape(G, P, T).transpose(1, 0, 2)
        ).astype(NP_BF16)                                        # [p, g, t]
        in_maps.append({
            "xT": xT, "q8": q8, "rT": rT, "lT": lT, "biasr": biasr,
        })
    return in_maps


_NC_CACHE = {}


def _get_nc():
    if "nc" not in _NC_CACHE:
        _NC_CACHE["nc"] = build_nc()
    return _NC_CACHE["nc"]


def run(inputs, trace=False, tmpdir=None):
    nc = _get_nc()
    in_maps = make_in_maps(**inputs)
    res = run_bass_kernel_spmd(
        nc, in_maps, list(range(N_CORES)), trace=trace, tmpdir=tmpdir
    )
    shards = [np.asarray(res.results[i]["out"]) for i in range(N_CORES)]
    full = np.concatenate(shards, axis=0).reshape(B, S, D_OUT)
    return full.astype(np.float32), res


def kernel(**inputs) -> np.ndarray:
    out, _ = run(inputs, trace=False)
    return out


# revision 10
# speedup vs baseline: 1.1446x; 1.1446x over previous
"""CalderaLinear Trainium2 kernel (v2: fp8 DoubleRow main GEMM).

Computes out = x @ dequant(q).T + (x @ dequant(r).T) @ dequant(l).T + bias
with groupwise (group=128) dequantization, distributed over 8 NeuronCores
by sharding tokens (batch*seq) 8 ways and replicating the weights.

Numerics: the output scale is dominated by the low-rank path (|out| up to
~1.4e6 vs main-path contributions ~2e3), so the x@q.T GEMM runs in fp8
(e4m3) with DoubleRow perf mode (2 fp8 weights/PE cell) at ~1.5-2x bf16
throughput while adding negligible absolute error. The low-rank path
(x@r.T and xr@l.T) stays bf16 with fp32 PSUM accumulation.

Host does layout only: dequant-multiply + transpose + fp8/bf16 casts and
token sharding; all 309 GFLOP of matmul work runs on device.

Device per core (1024 tokens):
  phase 1: stream x.T (bf16, 4 quarters), cast to fp8 copy, and compute
           xr.T = (x @ r_deq.T).T via 128 bf16 matmuls.
  phase 2: for each pair of 512-wide output blocks: stream q fp8 block,
           16 DoubleRow matmuls (256k x 128t x 512o) per psum tile pair
           sharing each stationary x tile, + 2 bf16 low-rank matmuls into
           the same psum group; bias added during the PSUM->SBUF copy.
"""

import os
import sys

import numpy as np
import ml_dtypes

for _p in ("/opt/trn_rl_repo",):
    if _p not in sys.path and os.path.isdir(_p):
        sys.path.insert(0, _p)

import concourse.bass as bass
import concourse.mybir as mybir
import concourse.tile as tile
from concourse import bacc
from concourse.bass_utils import run_bass_kernel_spmd

BF16 = mybir.dt.bfloat16
F32 = mybir.dt.float32
FP8 = mybir.dt.float8e4
NP_FP8 = ml_dtypes.float8_e4m3
NP_BF16 = ml_dtypes.bfloat16

P = 128  # partitions / dequant group size
N_CORES = 8

# Full problem shape (hardcoded per contest contract).
B, S, D_IN, D_OUT, RANK = 4, 2048, 4096, 4096, 256
N_TOK = B * S          # 8192
T = N_TOK // N_CORES   # 1024 tokens per core
G = D_IN // P          # 32 k-chunks
GP = G // 2            # 16 k-pair-chunks (DoubleRow)
OBW = 512              # output block width
NOB = D_OUT // OBW     # 8 output blocks
RC = RANK // P         # 2 rank chunks
XQ = 8                 # x streamed in XQ chunks
GQ = G // XQ           # k-chunks per x chunk


def caldera_kernel(tc, out, xT_d, q8_d, rT_d, lT_d, biasr_d):
    """One core. DRAM tensors:
    xT_d    [128, G, T]        bf16  xT[p,g,t] = x[t, g*128+p]
    q8_d    [NOB, 128, GP*2*OBW] fp8 q8[ob,p,gp,i,o] = qdeq[(2gp+i)*128+p, ob*512+o]
    rT_d    [128, G, RANK]     bf16  rT[p,g,r] = rdeq[r, g*128+p]
    lT_d    [128, RC, D_OUT]   bf16  lT[p,c,o] = ldeq[o, c*128+p]
    biasr_d [128, D_OUT]       f32   bias replicated over partitions
    out     [T, D_OUT]         f32
    """
    nc = tc.nc
    DR = mybir.MatmulPerfMode.DoubleRow
    NT = T // P  # 8 token tiles

    with tc.tile_pool(name="const", bufs=1) as constp, \
         tc.tile_pool(name="xq", bufs=2) as xqp, \
         tc.tile_pool(name="qs", bufs=4) as qsp, \
         tc.tile_pool(name="outp", bufs=4) as outp, \
         tc.tile_pool(name="ps", bufs=6, space="PSUM") as psp:

        # ---- resident tensors ----
        x8 = constp.tile([P, G, T], FP8)        # fp8 copy of xT
        rT = constp.tile([P, G, RANK], BF16)
        lT = constp.tile([P, RC, D_OUT], BF16)
        biasr = constp.tile([P, D_OUT], BF16)
        xrT = constp.tile([P, RC, T], BF16)     # xr.T chunks

        # Startup-critical DMAs first (emission order sets fetch priority):
        # first x chunk -> r weights -> first q pair; everything else after.
        xr_ps = [
            psp.tile([P, OBW], F32, tag="ps", name=f"xrps{i}")
            for i in range(4)
        ]
        xtiles = []
        for q4 in range(XQ):
            xq = xqp.tile([P, GQ, T], BF16, tag="xq", name=f"xq{q4}")
            xtiles.append(xq)

        nc.sync.dma_start(out=xtiles[0][:], in_=xT_d[:, 0:GQ, :])
        nc.sync.dma_start(out=rT[:], in_=rT_d[:])
        for q4 in range(1, XQ):
            nc.sync.dma_start(
                out=xtiles[q4][:], in_=xT_d[:, q4 * GQ:(q4 + 1) * GQ, :]
            )

        qtiles = []
        for ob in range(NOB):
            qt = qsp.tile([P, GP, 2, OBW], FP8, tag="q8b", name=f"q8b{ob}")
            qtiles.append(qt)

        def fetch_q(ob):
            nc.sync.dma_start(
                out=qtiles[ob][:].rearrange("p a b c -> p (a b c)"),
                in_=q8_d[ob],
            )

        fetch_q(0)
        fetch_q(1)
        nc.sync.dma_start(out=lT[:], in_=lT_d[:])
        nc.sync.dma_start(out=biasr[:], in_=biasr_d[:])

        # ---- x chunks: cast to fp8, feed xr matmuls ----
        for q4 in range(XQ):
            xq = xtiles[q4]
            # fp8 cast (vector engine), one op per chunk
            nc.vector.tensor_copy(
                out=x8[:, q4 * GQ:(q4 + 1) * GQ, :], in_=xq[:]
            )
            # xr.T accumulation: psum[rb*2+th] += rT-chunk.T @ xq-chunk
            for gl in range(GQ):
                g = q4 * GQ + gl
                for rb in range(RC):
                    for th in range(2):
                        nc.tensor.matmul(
                            xr_ps[rb * 2 + th][:],
                            lhsT=rT[:, g, rb * P:(rb + 1) * P],
                            rhs=xq[:, gl, th * OBW:(th + 1) * OBW],
                            start=(g == 0),
                            stop=(g == G - 1),
                        )
        for ob in range(2, NOB):
            fetch_q(ob)
        for rb in range(RC):
            for th in range(2):
                nc.scalar.copy(
                    xrT[:, rb, th * OBW:(th + 1) * OBW],
                    xr_ps[rb * 2 + th][:],
                )

        # ---- main: fp8 DoubleRow pairs + bf16 low-rank into same psum ----
        # Two output blocks share each stationary x tile: the DoubleRow
        # weight-load path (256 cols, no FWL) only keeps up when it has
        # 2 matmuls' worth of streaming time per unique stationary tile.
        for obp in range(NOB // 2):
            obA, obB = 2 * obp, 2 * obp + 1
            qA, qB = qtiles[obA], qtiles[obB]
            for t in range(NT):
                psA = psp.tile([P, OBW], F32, tag="ps", name=f"psA{obp}_{t}")
                psB = psp.tile([P, OBW], F32, tag="ps", name=f"psB{obp}_{t}")
                for gp in range(GP):
                    lhs = x8[:, 2 * gp:2 * gp + 2, t * P:(t + 1) * P]
                    nc.tensor.matmul(
                        psA[:], lhsT=lhs, rhs=qA[:, gp],
                        start=(gp == 0), stop=False, perf_mode=DR,
                    )
                    nc.tensor.matmul(
                        psB[:], lhsT=lhs, rhs=qB[:, gp],
                        start=(gp == 0), stop=False, perf_mode=DR,
                    )
                for rb in range(RC):
                    lhs2 = xrT[:, rb, t * P:(t + 1) * P]
                    nc.tensor.matmul(
                        psA[:], lhsT=lhs2,
                        rhs=lT[:, rb, obA * OBW:(obA + 1) * OBW],
                        start=False, stop=(rb == RC - 1),
                    )
                    nc.tensor.matmul(
                        psB[:], lhsT=lhs2,
                        rhs=lT[:, rb, obB * OBW:(obB + 1) * OBW],
                        start=False, stop=(rb == RC - 1),
                    )
                for ps, ob in ((psA, obA), (psB, obB)):
                    ot = outp.tile([P, OBW], F32, tag="ot")
                    nc.vector.tensor_tensor(
                        out=ot[:], in0=ps[:],
                        in1=biasr[:, ob * OBW:(ob + 1) * OBW],
                        op=mybir.AluOpType.add,
                    )
                    nc.sync.dma_start(
                        out=out[t * P:(t + 1) * P, ob * OBW:(ob + 1) * OBW],
                        in_=ot[:],
                    )


def build_nc():
    nc = bacc.Bacc("TRN2", target_bir_lowering=False, debug=False)
    xT_d = nc.dram_tensor("xT", [P, G, T], BF16, kind="ExternalInput").ap()
    q8_d = nc.dram_tensor(
        "q8", [NOB, P, GP * 2 * OBW], FP8, kind="ExternalInput"
    ).ap()
    rT_d = nc.dram_tensor("rT", [P, G, RANK], BF16, kind="ExternalInput").ap()
    lT_d = nc.dram_tensor("lT", [P, RC, D_OUT], BF16, kind="ExternalInput").ap()
    biasr_d = nc.dram_tensor(
        "biasr", [P, D_OUT], BF16, kind="ExternalInput"
    ).ap()
    out = nc.dram_tensor("out", [T, D_OUT], F32, kind="ExternalOutput").ap()
    with tile.TileContext(nc) as tc:
        caldera_kernel(tc, out, xT_d, q8_d, rT_d, lT_d, biasr_d)
    nc.compile()
    return nc


def _dequant(vals, scales):
    rows, cols = vals.shape
    g = cols // P
    v = vals.astype(np.float32).reshape(rows, g, P) * scales[:, :, None]
    return v.reshape(rows, cols)


def make_in_maps(x, q_values, q_scales, l_values, l_scales, r_values, r_scales,
                 bias):
    # q: dequant -> [k, o] transpose -> fp8, packed per 512-col block:
    # q8[ob, p, (gp, i, o)] = qdeq[(2gp+i)*128+p, ob*512+o]
    qdeq = _dequant(np.asarray(q_values), np.asarray(q_scales))  # [o, k]
    qT = np.ascontiguousarray(qdeq.T).astype(NP_FP8)             # [k, o]
    # [k, o] -> (gp, i, p, o) -> [p, gp, i, ob, o'] -> [ob, p, gp*i*o']
    q8 = qT.reshape(GP, 2, P, NOB, OBW).transpose(3, 2, 0, 1, 4)
    q8 = np.ascontiguousarray(q8).reshape(NOB, P, GP * 2 * OBW)

    rdeq = _dequant(np.asarray(r_values), np.asarray(r_scales))  # [r, k]
    rT = np.ascontiguousarray(
        rdeq.T.reshape(G, P, RANK).transpose(1, 0, 2)
    ).astype(NP_BF16)                                            # [p, g, r]

    ldeq = _dequant(np.asarray(l_values), np.asarray(l_scales))  # [o, r]
    lT = np.ascontiguousarray(
        ldeq.T.reshape(RC, P, D_OUT).transpose(1, 0, 2)
    ).astype(NP_BF16)                                            # [p, c, o]

    biasr = np.ascontiguousarray(
        np.broadcast_to(
            np.asarray(bias, dtype=np.float32).astype(NP_BF16), (P, D_OUT)
        )
    )

    xf = np.asarray(x, dtype=np.float32).reshape(N_TOK, D_IN)
    in_maps = []
    for i in range(N_CORES):
        xs = xf[i * T:(i + 1) * T]                               # [t, k]
        xT = np.ascontiguousarray(
            xs.T.reshape(G, P, T).transpose(1, 0, 2)
        ).astype(NP_BF16)                                        # [p, g, t]
        in_maps.append({
            "xT": xT, "q8": q8, "rT": rT, "lT": lT, "biasr": biasr,
        })
    return in_maps


_NC_CACHE = {}


def _get_nc():
    if "nc" not in _NC_CACHE:
        _NC_CACHE["nc"] = build_nc()
    return _NC_CACHE["nc"]


def run(inputs, trace=False, tmpdir=None):
    nc = _get_nc()
    in_maps = make_in_maps(**inputs)
    res = run_bass_kernel_spmd(
        nc, in_maps, list(range(N_CORES)), trace=trace, tmpdir=tmpdir
    )
    shards = [np.asarray(res.results[i]["out"]) for i in range(N_CORES)]
    full = np.concatenate(shards, axis=0).reshape(B, S, D_OUT)
    return full.astype(np.float32), res


def kernel(**inputs) -> np.ndarray:
    out, _ = run(inputs, trace=False)
    return out


# revision 12
# speedup vs baseline: 1.1663x; 1.0189x over previous
"""CalderaLinear Trainium2 kernel (v2: fp8 DoubleRow main GEMM).

Computes out = x @ dequant(q).T + (x @ dequant(r).T) @ dequant(l).T + bias
with groupwise (group=128) dequantization, distributed over 8 NeuronCores
by sharding tokens (batch*seq) 8 ways and replicating the weights.

Numerics: the output scale is dominated by the low-rank path (|out| up to
~1.4e6 vs main-path contributions ~2e3), so the x@q.T GEMM runs in fp8
(e4m3) with DoubleRow perf mode (2 fp8 weights/PE cell) at ~1.5-2x bf16
throughput while adding negligible absolute error. The low-rank path
(x@r.T and xr@l.T) stays bf16 with fp32 PSUM accumulation.

Host does layout only: dequant-multiply + transpose + fp8/bf16 casts and
token sharding; all 309 GFLOP of matmul work runs on device.

Device per core (1024 tokens):
  phase 1: stream x.T (bf16, 4 quarters), cast to fp8 copy, and compute
           xr.T = (x @ r_deq.T).T via 128 bf16 matmuls.
  phase 2: for each pair of 512-wide output blocks: stream q fp8 block,
           16 DoubleRow matmuls (256k x 128t x 512o) per psum tile pair
           sharing each stationary x tile, + 2 bf16 low-rank matmuls into
           the same psum group; bias added during the PSUM->SBUF copy.
"""

import os
import sys

import numpy as np
import ml_dtypes

for _p in ("/opt/trn_rl_repo",):
    if _p not in sys.path and os.path.isdir(_p):
        sys.path.insert(0, _p)

import concourse.bass as bass
import concourse.mybir as mybir
import concourse.tile as tile
from concourse import bacc
from concourse.bass_utils import run_bass_kernel_spmd

BF16 = mybir.dt.bfloat16
F32 = mybir.dt.float32
FP8 = mybir.dt.float8e4
NP_FP8 = ml_dtypes.float8_e4m3
NP_BF16 = ml_dtypes.bfloat16

P = 128  # partitions / dequant group size
N_CORES = 8

# Full problem shape (hardcoded per contest contract).
B, S, D_IN, D_OUT, RANK = 4, 2048, 4096, 4096, 256
N_TOK = B * S          # 8192
T = N_TOK // N_CORES   # 1024 tokens per core
G = D_IN // P          # 32 k-chunks
GP = G // 2            # 16 k-pair-chunks (DoubleRow)
OBW = 512              # output block width
NOB = D_OUT // OBW     # 8 output blocks
RC = RANK // P         # 2 rank chunks
XQ = 8                 # x streamed in XQ chunks
GQ = G // XQ           # k-chunks per x chunk


def caldera_kernel(tc, out, xT_d, q8_d, rT_d, lT_d, biasr_d):
    """One core. DRAM tensors:
    xT_d    [128, G, T]        bf16  xT[p,g,t] = x[t, g*128+p]
    q8_d    [NOB, 128, GP*2*OBW] fp8 q8[ob,p,gp,i,o] = qdeq[(2gp+i)*128+p, ob*512+o]
    rT_d    [128, G, RANK]     bf16  rT[p,g,r] = rdeq[r, g*128+p]
    lT_d    [128, RC, D_OUT]   bf16  lT[p,c,o] = ldeq[o, c*128+p]
    biasr_d [128, D_OUT]       f32   bias replicated over partitions
    out     [T, D_OUT]         f32
    """
    nc = tc.nc
    DR = mybir.MatmulPerfMode.DoubleRow
    NT = T // P  # 8 token tiles

    with tc.tile_pool(name="const", bufs=1) as constp, \
         tc.tile_pool(name="xq", bufs=2) as xqp, \
         tc.tile_pool(name="qs", bufs=5) as qsp, \
         tc.tile_pool(name="outp", bufs=6) as outp, \
         tc.tile_pool(name="ps", bufs=8, space="PSUM") as psp:

        # ---- resident tensors ----
        x8 = constp.tile([P, G, T], FP8)        # fp8 copy of xT
        rT = constp.tile([P, G, RANK], BF16)
        lT = constp.tile([P, RC, D_OUT], BF16)
        biasr = constp.tile([P, D_OUT], BF16)
        xrT = constp.tile([P, RC, T], BF16)     # xr.T chunks

        # Startup-critical DMAs first (emission order sets fetch priority):
        # first x chunk -> r weights -> first q pair; everything else after.
        xr_ps = [
            psp.tile([P, OBW], F32, tag="ps", name=f"xrps{i}")
            for i in range(4)
        ]
        xtiles = []
        for q4 in range(XQ):
            xq = xqp.tile([P, GQ, T], BF16, tag="xq", name=f"xq{q4}")
            xtiles.append(xq)

        qtiles = []
        for ob in range(NOB):
            qt = qsp.tile([P, GP, 2, OBW], FP8, tag="q8b", name=f"q8b{ob}")
            qtiles.append(qt)

        def fetch_q(ob):
            nc.sync.dma_start(
                out=qtiles[ob][:].rearrange("p a b c -> p (a b c)"),
                in_=q8_d[ob],
            )

        def fetch_x(q4):
            nc.sync.dma_start(
                out=xtiles[q4][:], in_=xT_d[:, q4 * GQ:(q4 + 1) * GQ, :]
            )

        # Startup-critical DMA order: first x chunk + first q block let the
        # scheduler interleave early main-GEMM matmuls into the DMA-paced
        # xr phase; lT/biasr are only needed ~35us in.
        fetch_x(0)
        fetch_q(0)
        nc.sync.dma_start(out=rT[:], in_=rT_d[:])
        fetch_x(1)
        fetch_q(1)
        fetch_x(2)
        fetch_x(3)
        nc.sync.dma_start(out=lT[:], in_=lT_d[:])
        for q4 in range(4, XQ):
            fetch_x(q4)
        nc.sync.dma_start(out=biasr[:], in_=biasr_d[:])

        # ---- x chunks: cast to fp8, feed xr matmuls ----
        for q4 in range(XQ):
            xq = xtiles[q4]
            # fp8 cast (vector engine), one op per chunk
            nc.vector.tensor_copy(
                out=x8[:, q4 * GQ:(q4 + 1) * GQ, :], in_=xq[:]
            )
            # xr.T accumulation: psum[rb*2+th] += rT-chunk.T @ xq-chunk
            for gl in range(GQ):
                g = q4 * GQ + gl
                for rb in range(RC):
                    for th in range(2):
                        nc.tensor.matmul(
                            xr_ps[rb * 2 + th][:],
                            lhsT=rT[:, g, rb * P:(rb + 1) * P],
                            rhs=xq[:, gl, th * OBW:(th + 1) * OBW],
                            start=(g == 0),
                            stop=(g == G - 1),
                        )
        for ob in range(2, NOB):
            fetch_q(ob)
        for rb in range(RC):
            for th in range(2):
                nc.scalar.copy(
                    xrT[:, rb, th * OBW:(th + 1) * OBW],
                    xr_ps[rb * 2 + th][:],
                )

        # ---- main: fp8 DoubleRow pairs + bf16 low-rank into same psum ----
        # Two output blocks share each stationary x tile: the DoubleRow
        # weight-load path (256 cols, no FWL) only keeps up when it has
        # 2 matmuls' worth of streaming time per unique stationary tile.
        for obp in range(NOB // 2):
            obA, obB = 2 * obp, 2 * obp + 1
            qA, qB = qtiles[obA], qtiles[obB]
            for t in range(NT):
                psA = psp.tile([P, OBW], F32, tag="ps", name=f"psA{obp}_{t}")
                psB = psp.tile([P, OBW], F32, tag="ps", name=f"psB{obp}_{t}")
                for gp in range(GP):
                    lhs = x8[:, 2 * gp:2 * gp + 2, t * P:(t + 1) * P]
                    nc.tensor.matmul(
                        psA[:], lhsT=lhs, rhs=qA[:, gp],
                        start=(gp == 0), stop=False, perf_mode=DR,
                    )
                    nc.tensor.matmul(
                        psB[:], lhsT=lhs, rhs=qB[:, gp],
                        start=(gp == 0), stop=False, perf_mode=DR,
                    )
                for rb in range(RC):
                    lhs2 = xrT[:, rb, t * P:(t + 1) * P]
                    nc.tensor.matmul(
                        psA[:], lhsT=lhs2,
                        rhs=lT[:, rb, obA * OBW:(obA + 1) * OBW],
                        start=False, stop=(rb == RC - 1),
                    )
                    nc.tensor.matmul(
                        psB[:], lhsT=lhs2,
                        rhs=lT[:, rb, obB * OBW:(obB + 1) * OBW],
                        start=False, stop=(rb == RC - 1),
                    )
                for ps, ob in ((psA, obA), (psB, obB)):
                    ot = outp.tile([P, OBW], F32, tag="ot")
                    nc.vector.tensor_tensor(
                        out=ot[:], in0=ps[:],
                        in1=biasr[:, ob * OBW:(ob + 1) * OBW],
                        op=mybir.AluOpType.add,
                    )
                    nc.sync.dma_start(
                        out=out[t * P:(t + 1) * P, ob * OBW:(ob + 1) * OBW],
                        in_=ot[:],
                    )


def build_nc():
    nc = bacc.Bacc("TRN2", target_bir_lowering=False, debug=False)
    xT_d = nc.dram_tensor("xT", [P, G, T], BF16, kind="ExternalInput").ap()
    q8_d = nc.dram_tensor(
        "q8", [NOB, P, GP * 2 * OBW], FP8, kind="ExternalInput"
    ).ap()
    rT_d = nc.dram_tensor("rT", [P, G, RANK], BF16, kind="ExternalInput").ap()
    lT_d = nc.dram_tensor("lT", [P, RC, D_OUT], BF16, kind="ExternalInput").ap()
    biasr_d = nc.dram_tensor(
        "biasr", [P, D_OUT], BF16, kind="ExternalInput"
    ).ap()
    out = nc.dram_tensor("out", [T, D_OUT], F32, kind="ExternalOutput").ap()
    with tile.TileContext(nc) as tc:
        caldera_kernel(tc, out, xT_d, q8_d, rT_d, lT_d, biasr_d)
    nc.compile()
    return nc


def _dequant(vals, scales):
    rows, cols = vals.shape
    g = cols // P
    v = vals.astype(np.float32).reshape(rows, g, P) * scales[:, :, None]
    return v.reshape(rows, cols)


def make_in_maps(x, q_values, q_scales, l_values, l_scales, r_values, r_scales,
                 bias):
    # q: dequant -> [k, o] transpose -> fp8, packed per 512-col block:
    # q8[ob, p, (gp, i, o)] = qdeq[(2gp+i)*128+p, ob*512+o]
    qdeq = _dequant(np.asarray(q_values), np.asarray(q_scales))  # [o, k]
    qT = np.ascontiguousarray(qdeq.T).astype(NP_FP8)             # [k, o]
    # [k, o] -> (gp, i, p, o) -> [p, gp, i, ob, o'] -> [ob, p, gp*i*o']
    q8 = qT.reshape(GP, 2, P, NOB, OBW).transpose(3, 2, 0, 1, 4)
    q8 = np.ascontiguousarray(q8).reshape(NOB, P, GP * 2 * OBW)

    rdeq = _dequant(np.asarray(r_values), np.asarray(r_scales))  # [r, k]
    rT = np.ascontiguousarray(
        rdeq.T.reshape(G, P, RANK).transpose(1, 0, 2)
    ).astype(NP_BF16)                                            # [p, g, r]

    ldeq = _dequant(np.asarray(l_values), np.asarray(l_scales))  # [o, r]
    lT = np.ascontiguousarray(
        ldeq.T.reshape(RC, P, D_OUT).transpose(1, 0, 2)
    ).astype(NP_BF16)                                            # [p, c, o]

    biasr = np.ascontiguousarray(
        np.broadcast_to(
            np.asarray(bias, dtype=np.float32).astype(NP_BF16), (P, D_OUT)
        )
    )

    xf = np.asarray(x, dtype=np.float32).reshape(N_TOK, D_IN)
    in_maps = []
    for i in range(N_CORES):
        xs = xf[i * T:(i + 1) * T]                               # [t, k]
        xT = np.ascontiguousarray(
            xs.T.reshape(G, P, T).transpose(1, 0, 2)
        ).astype(NP_BF16)                                        # [p, g, t]
        in_maps.append({
            "xT": xT, "q8": q8, "rT": rT, "lT": lT, "biasr": biasr,
        })
    return in_maps


_NC_CACHE = {}


def _get_nc():
    if "nc" not in _NC_CACHE:
        _NC_CACHE["nc"] = build_nc()
    return _NC_CACHE["nc"]


def run(inputs, trace=False, tmpdir=None):
    nc = _get_nc()
    in_maps = make_in_maps(**inputs)
    res = run_bass_kernel_spmd(
        nc, in_maps, list(range(N_CORES)), trace=trace, tmpdir=tmpdir
    )
    shards = [np.asarray(res.results[i]["out"]) for i in range(N_CORES)]
    full = np.concatenate(shards, axis=0).reshape(B, S, D_OUT)
    return full.astype(np.float32), res


def kernel(**inputs) -> np.ndarray:
    out, _ = run(inputs, trace=False)
    return out
